# revision 19
# baseline (speedup 1.0000x reference)
"""Trainium2 Bass kernel for nn_MultiHeadAttention_79130477461654.

The reference einsum "nhqk,nhvd->nhqd" contracts k and v independently, so
out = (sum_k softmax(energy))*(sum_s v) = broadcast(sum_s v) since softmax
rows sum to 1.  With v = split_heads(x @ Wv) and the reference's direct
(n,h,q,d)->(n,s,e) reshape, the full output reduces to

    xs[n]    = sum_s x[n,s,:]                       (1024,)
    Z[n]     = xs[n] @ Wv                           (1024,)
    WoSum    = sum_m Wo[64m+d, :]  (d=0..63)        (64, 1024)
    T[n,h,:] = Z[n][64h:64h+64] @ WoSum + bo        (16, 1024)
    out[n, 64h+r, :] = T[n,h,:]   for r in 0..63

Sharding: data parallel over batch N=8, one batch per core; weights
replicated.  All arithmetic on-device.

v13 = v2 (the empirical best at 41.0us) + one fix: v2's K=1 bias matmul
head-of-line-blocked the PE queue until ~15us waiting on the slow SWDGE
const DMA for bo.  bo is now uploaded host-pre-broadcast as a [128,1024]
tile on the SP ring (a normal 256 KB stream item), the bias matmuls are
gone, and the bias is added by DVE during the PSUM->SBUF tb8 copy.

v2 structure: all-bf16 streams; x-chunk k and Wv-chunk k adjacent on
ring k%2 so the Z accumulation chases the stream; Wo last as two
column-halves (4 staggered 512 KB sub-tiles) with the WoSum row-fold
fused into the T matmuls; Z->srow on ACT (bf16, single-pass rank-1
transposes); out as two column-half broadcast DMAs.
"""

import numpy as np

N, S, E, H, D = 8, 1024, 1024, 16, 64
NCORES = 8
P = 128  # partitions
NCHUNK = 8  # 1024 rows / 128


def build_nc():
    import concourse.bacc as bacc
    import concourse.mybir as mybir
    from concourse.tile import TileContext

    F32 = mybir.dt.float32
    BF16 = mybir.dt.bfloat16
    nc = bacc.Bacc("TRN2", target_bir_lowering=False, debug=False)

    xtd = nc.declare_dram_parameter("xT", [E, S], BF16, isOutput=False)
    wvd = nc.declare_dram_parameter("Wv", [E, E], BF16, isOutput=False)
    # Wo re-laid-out on host as two contiguous column halves: [2048, 512]
    wod = nc.declare_dram_parameter("WoH", [2 * E, E // 2], BF16, isOutput=False)
    bod = nc.declare_dram_parameter("bo128", [P, E], BF16, isOutput=False)
    i2d = nc.declare_dram_parameter("I2", [D, P], BF16, isOutput=False)
    outd = nc.declare_dram_parameter("out", [S, E], BF16, isOutput=True)

    # two HWDGE queues: SP (sync) and ACT (scalar)
    dmae = [nc.sync, nc.scalar]
    Copy = mybir.ActivationFunctionType.Copy

    with TileContext(nc) as tc:
        with (
            tc.tile_pool(name="xin", bufs=NCHUNK) as xp,
            tc.tile_pool(name="wv", bufs=NCHUNK) as wvp,
            tc.tile_pool(name="wo", bufs=4) as wop,
            tc.tile_pool(name="small", bufs=1) as sp,
            tc.tile_pool(name="psZ", bufs=1, space="PSUM") as psZ,
            tc.tile_pool(name="psS", bufs=1, space="PSUM") as psS,
            tc.tile_pool(name="psY", bufs=1, space="PSUM") as psY,
            tc.tile_pool(name="psT", bufs=1, space="PSUM") as psT,
        ):
            # I2 on the SWDGE queue (needed only at the dup matmul, slow
            # SWDGE small-transfer latency is fine for it)
            i2_sb = sp.tile([D, P], BF16)
            nc.gpsimd.dma_start(out=i2_sb[:], in_=i2d[:])
            ones18 = sp.tile([1, 8], BF16)
            nc.vector.memset(ones18[:], 1.0)

            # ---- input DMAs: x/Wv as 8 256KB chunk tiles each, chunk k of
            #      x and Wv adjacent on queue k%2 so Z-chunk matmuls fire
            #      throughout the stream; bo128 then Wo last (two column
            #      halves, each split into rb-groups 0-3 / 4-7).
            xr = xtd.rearrange("(k p) s -> k p s", p=P)
            wr = wvd.rearrange("(k p) e -> k p e", p=P)
            wor = wod.rearrange("(t rb p) c -> t p rb c", rb=4, p=P)
            xts = [None] * NCHUNK
            wvt = [None] * NCHUNK
            for k in range(NCHUNK):
                t = xp.tile([P, S], BF16, tag="xt")
                dmae[k % 2].dma_start(out=t[:], in_=xr[k])
                xts[k] = t
                t = wvp.tile([P, E], BF16, tag="wv")
                dmae[k % 2].dma_start(out=t[:], in_=wr[k])
                wvt[k] = t
            wot = [None] * 4
            bo_sb = sp.tile([P, E], BF16)
            for i in range(4):
                # i = 0,1 -> column half A (rb 0-3, 4-7); i = 2,3 -> half B.
                # halves split across both queues so half A lands first;
                # bo128 rides ring0 between the halves (needed only at tb8).
                t = wop.tile([P, 4 * (E // 2)], BF16, tag="wo")
                dmae[i % 2].dma_start(
                    out=t[:].rearrange("p (rb c) -> p rb c", rb=4), in_=wor[i]
                )
                wot[i] = t
                if i == 1:
                    dmae[0].dma_start(out=bo_sb[:], in_=bod[:])

            # ---- DVE: per-chunk seq-sum of x straight to bf16 (fp32
            #      internal accumulation on DVE): xpb[p, k] = sum_s x[128k+p, s]
            xpb = sp.tile([P, NCHUNK], BF16)
            with nc.allow_low_precision(
                reason="reduce accumulates fp32 internally; bf16 only on write"
            ):
                for k in range(NCHUNK):
                    if k % 2 == 0:
                        nc.vector.tensor_reduce(
                            xpb[:, k : k + 1],
                            xts[k][:],
                            axis=mybir.AxisListType.X,
                            op=mybir.AluOpType.add,
                        )
                    else:
                        # GpSimd pre-folds odd chunks 2:1 so DVE keeps up
                        fk = sp.tile([P, S // 2], BF16, tag="fold")
                        nc.gpsimd.tensor_add(
                            fk[:], xts[k][:, 0 : S // 2], xts[k][:, S // 2 : S]
                        )
                        nc.vector.tensor_reduce(
                            xpb[:, k : k + 1],
                            fk[:],
                            axis=mybir.AxisListType.X,
                            op=mybir.AluOpType.add,
                        )

            # ---- Z row (1, 1024) = xs @ Wv, accumulated chunk by chunk as
            #      the stream delivers (x_k, Wv_k); bf16 single-pass.
            ps_z = psZ.tile([1, E], F32, tag="psz")
            for k in range(NCHUNK):
                for half in range(2):
                    sl = slice(half * 512, half * 512 + 512)
                    nc.tensor.matmul(
                        ps_z[0:1, sl],
                        xpb[:, k : k + 1],
                        wvt[k][:, sl],
                        start=(k == 0),
                        stop=(k == NCHUNK - 1),
                        skip_group_check=True,
                    )

            # ---- gap filler: keep the PE busy while ACT copies srow so
            #      HAM stays at 8/8 into the dance and T chain
            ps_ytx = psY.tile([P, P], F32, tag="psy")
            onescol = sp.tile([P, 1], BF16)
            nc.vector.memset(onescol[:], 1.0)
            for f in range(10):
                nc.tensor.matmul(
                    ps_ytx[0:1, :],
                    onescol[:, 0:1],
                    wvt[0][:, (f % 8) * P : (f % 8) * P + P],
                    start=True,
                    stop=True,
                    skip_group_check=True,
                )

            # ---- Z -> srow (bf16, on ACT so DVE stays free and the rank-1
            #      transposes below run single-pass bf16)
            srow = sp.tile([1, E], BF16)
            for half in range(2):
                sl = slice(half * 512, half * 512 + 512)
                nc.scalar.activation(
                    srow[0:1, sl], ps_z[0:1, sl], func=Copy,
                )

            # ---- transpose dance: ps_sft[d, 8h+rr] = Z[64h+d] via 16
            #      rank-1 matmuls (rhs = ones[1,8] replicates over rr)
            ps_sft = psS.tile([D, P], F32, tag="pss")
            for h in range(H):
                nc.tensor.matmul(
                    ps_sft[:, 8 * h : 8 * h + 8],
                    srow[0:1, h * D : (h + 1) * D],
                    ones18[0:1, :],
                    start=True,
                    stop=True,
                    skip_group_check=True,
                )
            # gap filler: cover the sft8 cast window on DVE
            for f in range(6):
                nc.tensor.matmul(
                    ps_ytx[0:1, :],
                    onescol[:, 0:1],
                    wvt[0][:, (f % 8) * P : (f % 8) * P + P],
                    start=True,
                    stop=True,
                    skip_group_check=True,
                )
            sft8 = sp.tile([D, P], BF16)
            nc.vector.tensor_copy(sft8[:], ps_sft[:])
            # dup matmul: ytx8[p, m] = sft8[p%64, m]  (I2[d,p]=1 iff d==p%64)
            nc.tensor.matmul(
                ps_ytx[:], i2_sb[:], sft8[:], start=True, stop=True,
                skip_group_check=True,
            )
            ytx8 = sp.tile([P, P], BF16)
            nc.vector.tensor_copy(ytx8[:], ps_ytx[:])

            # ---- T accumulation fused with the Wo row-fold: for column
            #      half, psT[:, half] = sum_rb ytx8 @ Wo[128rb+p, half].
            #      Chases the Wo stream tile by tile; then DVE bias-add
            #      (bf16 out) and the broadcast store
            #      out[8m + r8, half] = tb8[m, half].
            ps_t = psT.tile([P, E], F32, tag="pst")
            tb8 = sp.tile([P, E], BF16)
            outr = outd.rearrange("(m r8) e -> m r8 e", r8=8)
            for half in range(2):
                sl = slice(half * 512, half * 512 + 512)
                for i in (0, 1):
                    wt = wot[2 * half + i]
                    for rb in range(4):
                        nc.tensor.matmul(
                            ps_t[:, sl],
                            ytx8[:],
                            wt[:, rb * 512 : rb * 512 + 512],
                            start=(i == 0 and rb == 0),
                            stop=(i == 1 and rb == 3),
                            skip_group_check=True,
                        )
                nc.vector.tensor_add(tb8[:, sl], ps_t[:, sl], bo_sb[:, sl])
                dmae[half].dma_start(
                    out=outr[:, :, sl],
                    in_=tb8[:, None, sl].to_broadcast((P, 8, 512)),
                )

    nc.compile()
    return nc


_NC_CACHE = None


def make_in_maps(x, Wv, Wo, bo):
    import ml_dtypes

    BF = ml_dtypes.bfloat16
    x = np.asarray(x, dtype=np.float32)
    Wv = np.ascontiguousarray(np.asarray(Wv, dtype=np.float32).astype(BF))
    Wo = np.asarray(Wo, dtype=np.float32).astype(BF)
    WoH = np.ascontiguousarray(np.concatenate([Wo[:, :512], Wo[:, 512:]], axis=0))
    bo128 = np.ascontiguousarray(
        np.broadcast_to(np.asarray(bo, dtype=np.float32).astype(BF), (P, E))
    )
    I2 = np.zeros((D, P), dtype=BF)
    I2[np.arange(P) % D, np.arange(P)] = 1.0
    return [
        {
            "xT": np.ascontiguousarray(x[j].T.astype(BF)),
            "Wv": Wv,
            "WoH": WoH,
            "bo128": bo128,
            "I2": I2,
        }
        for j in range(NCORES)
    ]


def kernel(x, Wq=None, Wk=None, Wv=None, Wo=None, bo=None, **_unused):
    from concourse.bass_utils import run_bass_kernel_spmd

    global _NC_CACHE
    if _NC_CACHE is None:
        _NC_CACHE = build_nc()
    nc = _NC_CACHE

    in_maps = make_in_maps(x, Wv, Wo, bo)
    res = run_bass_kernel_spmd(nc, in_maps, core_ids=list(range(NCORES))).results
    return np.stack(
        [res[j]["out"].astype(np.float32) for j in range(NCORES)], axis=0
    )


# revision 20
# speedup vs baseline: 1.0215x; 1.0215x over previous
"""Trainium2 Bass kernel for nn_MultiHeadAttention_79130477461654.

The reference einsum "nhqk,nhvd->nhqd" contracts k and v independently, so
out = (sum_k softmax(energy))*(sum_s v) = broadcast(sum_s v) since softmax
rows sum to 1.  With v = split_heads(x @ Wv) and the reference's direct
(n,h,q,d)->(n,s,e) reshape, the full output reduces to

    xs[n]    = sum_s x[n,s,:]                       (1024,)
    Z[n]     = xs[n] @ Wv                           (1024,)
    WoSum    = sum_m Wo[64m+d, :]  (d=0..63)        (64, 1024)
    T[n,h,:] = Z[n][64h:64h+64] @ WoSum + bo        (16, 1024)
    out[n, 64h+r, :] = T[n,h,:]   for r in 0..63

Sharding: data parallel over batch N=8, one batch per core; weights
replicated.  All arithmetic on-device.

v13 = v2 (the empirical best at 41.0us) + one fix: v2's K=1 bias matmul
head-of-line-blocked the PE queue until ~15us waiting on the slow SWDGE
const DMA for bo.  bo is now uploaded host-pre-broadcast as a [128,1024]
tile on the SP ring (a normal 256 KB stream item), the bias matmuls are
gone, and the bias is added by DVE during the PSUM->SBUF tb8 copy.

v2 structure: all-bf16 streams; x-chunk k and Wv-chunk k adjacent on
ring k%2 so the Z accumulation chases the stream; Wo last as two
column-halves (4 staggered 512 KB sub-tiles) with the WoSum row-fold
fused into the T matmuls; Z->srow on ACT (bf16, single-pass rank-1
transposes); out as two column-half broadcast DMAs.
"""

import numpy as np

N, S, E, H, D = 8, 1024, 1024, 16, 64
NCORES = 8
P = 128  # partitions
NCHUNK = 8  # 1024 rows / 128


def build_nc():
    import concourse.bacc as bacc
    import concourse.mybir as mybir
    from concourse.tile import TileContext

    F32 = mybir.dt.float32
    BF16 = mybir.dt.bfloat16
    nc = bacc.Bacc("TRN2", target_bir_lowering=False, debug=False)

    xtd = nc.declare_dram_parameter("xT", [E, S], BF16, isOutput=False)
    wvd = nc.declare_dram_parameter("Wv", [E, E], BF16, isOutput=False)
    # Wo re-laid-out on host as two contiguous column halves: [2048, 512]
    wod = nc.declare_dram_parameter("WoH", [2 * E, E // 2], BF16, isOutput=False)
    bod = nc.declare_dram_parameter("bo128", [P, E], BF16, isOutput=False)
    i2d = nc.declare_dram_parameter("I2", [D, P], BF16, isOutput=False)
    outd = nc.declare_dram_parameter("out", [S, E], BF16, isOutput=True)

    # two HWDGE queues: SP (sync) and ACT (scalar)
    dmae = [nc.sync, nc.scalar]
    Copy = mybir.ActivationFunctionType.Copy

    with TileContext(nc) as tc:
        with (
            tc.tile_pool(name="xin", bufs=NCHUNK) as xp,
            tc.tile_pool(name="wv", bufs=NCHUNK) as wvp,
            tc.tile_pool(name="wo", bufs=4) as wop,
            tc.tile_pool(name="small", bufs=1) as sp,
            tc.tile_pool(name="psZ", bufs=1, space="PSUM") as psZ,
            tc.tile_pool(name="psS", bufs=1, space="PSUM") as psS,
            tc.tile_pool(name="psY", bufs=1, space="PSUM") as psY,
            tc.tile_pool(name="psT", bufs=1, space="PSUM") as psT,
        ):
            # I2 on the SWDGE queue (needed only at the dup matmul, slow
            # SWDGE small-transfer latency is fine for it)
            i2_sb = sp.tile([D, P], BF16)
            nc.gpsimd.dma_start(out=i2_sb[:], in_=i2d[:])
            ones18 = sp.tile([1, 8], BF16)
            nc.vector.memset(ones18[:], 1.0)

            # ---- input DMAs: x/Wv as 8 256KB chunk tiles each, chunk k of
            #      x and Wv adjacent on queue k%2 so Z-chunk matmuls fire
            #      throughout the stream; bo128 then Wo last (two column
            #      halves, each split into rb-groups 0-3 / 4-7).
            xr = xtd.rearrange("(k p) s -> k p s", p=P)
            wr = wvd.rearrange("(k p) e -> k p e", p=P)
            wor = wod.rearrange("(t rb p) c -> t p rb c", rb=4, p=P)
            xts = [None] * NCHUNK
            wvt = [None] * NCHUNK
            for k in range(NCHUNK):
                t = xp.tile([P, S], BF16, tag="xt")
                dmae[k % 2].dma_start(out=t[:], in_=xr[k])
                xts[k] = t
                t = wvp.tile([P, E], BF16, tag="wv")
                dmae[k % 2].dma_start(out=t[:], in_=wr[k])
                wvt[k] = t
            bo_sb = sp.tile([P, E], BF16)
            dmae[0].dma_start(out=bo_sb[:], in_=bod[:])
            wot = [None] * 4
            for i in range(4):
                # i = 0,1 -> column half A (rb 0-3, 4-7); i = 2,3 -> half B.
                # halves split across both queues so half A lands first.
                t = wop.tile([P, 4 * (E // 2)], BF16, tag="wo")
                dmae[i % 2].dma_start(
                    out=t[:].rearrange("p (rb c) -> p rb c", rb=4), in_=wor[i]
                )
                wot[i] = t

            # ---- DVE: per-chunk seq-sum of x straight to bf16 (fp32
            #      internal accumulation on DVE): xpb[p, k] = sum_s x[128k+p, s]
            xpb = sp.tile([P, NCHUNK], BF16)
            with nc.allow_low_precision(
                reason="DVE reduce accumulates fp32 internally; bf16 only on write"
            ):
                for k in range(NCHUNK):
                    nc.vector.tensor_reduce(
                        xpb[:, k : k + 1],
                        xts[k][:],
                        axis=mybir.AxisListType.X,
                        op=mybir.AluOpType.add,
                    )

            # ---- Z row (1, 1024) = xs @ Wv, accumulated chunk by chunk as
            #      the stream delivers (x_k, Wv_k); bf16 single-pass.
            ps_z = psZ.tile([1, E], F32, tag="psz")
            for k in range(NCHUNK):
                for half in range(2):
                    sl = slice(half * 512, half * 512 + 512)
                    nc.tensor.matmul(
                        ps_z[0:1, sl],
                        xpb[:, k : k + 1],
                        wvt[k][:, sl],
                        start=(k == 0),
                        stop=(k == NCHUNK - 1),
                        skip_group_check=True,
                    )

            # ---- Z -> srow (bf16, on ACT so DVE stays free and the rank-1
            #      transposes below run single-pass bf16)
            srow = sp.tile([1, E], BF16)
            for half in range(2):
                sl = slice(half * 512, half * 512 + 512)
                nc.scalar.activation(
                    srow[0:1, sl], ps_z[0:1, sl], func=Copy,
                )

            # ---- transpose dance: ps_sft[d, 8h+rr] = Z[64h+d] via 16
            #      rank-1 matmuls (rhs = ones[1,8] replicates over rr)
            ps_sft = psS.tile([D, P], F32, tag="pss")
            for h in range(H):
                nc.tensor.matmul(
                    ps_sft[:, 8 * h : 8 * h + 8],
                    srow[0:1, h * D : (h + 1) * D],
                    ones18[0:1, :],
                    start=True,
                    stop=True,
                    skip_group_check=True,
                )
            sft8 = sp.tile([D, P], BF16)
            nc.vector.tensor_copy(sft8[:], ps_sft[:])
            # dup matmul: ytx8[p, m] = sft8[p%64, m]  (I2[d,p]=1 iff d==p%64)
            ps_ytx = psY.tile([P, P], F32, tag="psy")
            nc.tensor.matmul(
                ps_ytx[:], i2_sb[:], sft8[:], start=True, stop=True,
                skip_group_check=True,
            )
            ytx8 = sp.tile([P, P], BF16)
            nc.vector.tensor_copy(ytx8[:], ps_ytx[:])

            # ---- T accumulation fused with the Wo row-fold: for column
            #      half, psT[:, half] = sum_rb ytx8 @ Wo[128rb+p, half].
            #      Chases the Wo stream tile by tile; then DVE bias-add
            #      (bf16 out) and the broadcast store
            #      out[8m + r8, half] = tb8[m, half].
            ps_t = psT.tile([P, E], F32, tag="pst")
            tb8 = sp.tile([P, E], BF16)
            outr = outd.rearrange("(m r8) e -> m r8 e", r8=8)
            for half in range(2):
                sl = slice(half * 512, half * 512 + 512)
                for i in (0, 1):
                    wt = wot[2 * half + i]
                    for rb in range(4):
                        nc.tensor.matmul(
                            ps_t[:, sl],
                            ytx8[:],
                            wt[:, rb * 512 : rb * 512 + 512],
                            start=(i == 0 and rb == 0),
                            stop=(i == 1 and rb == 3),
                            skip_group_check=True,
                        )
                nc.vector.tensor_add(tb8[:, sl], ps_t[:, sl], bo_sb[:, sl])
                dmae[half].dma_start(
                    out=outr[:, :, sl],
                    in_=tb8[:, None, sl].to_broadcast((P, 8, 512)),
                )

    nc.compile()
    return nc


_NC_CACHE = None


def make_in_maps(x, Wv, Wo, bo):
    import ml_dtypes

    BF = ml_dtypes.bfloat16
    x = np.asarray(x, dtype=np.float32)
    Wv = np.ascontiguousarray(np.asarray(Wv, dtype=np.float32).astype(BF))
    Wo = np.asarray(Wo, dtype=np.float32).astype(BF)
    WoH = np.ascontiguousarray(np.concatenate([Wo[:, :512], Wo[:, 512:]], axis=0))
    bo128 = np.ascontiguousarray(
        np.broadcast_to(np.asarray(bo, dtype=np.float32).astype(BF), (P, E))
    )
    I2 = np.zeros((D, P), dtype=BF)
    I2[np.arange(P) % D, np.arange(P)] = 1.0
    return [
        {
            "xT": np.ascontiguousarray(x[j].T.astype(BF)),
            "Wv": Wv,
            "WoH": WoH,
            "bo128": bo128,
            "I2": I2,
        }
        for j in range(NCORES)
    ]


def kernel(x, Wq=None, Wk=None, Wv=None, Wo=None, bo=None, **_unused):
    from concourse.bass_utils import run_bass_kernel_spmd

    global _NC_CACHE
    if _NC_CACHE is None:
        _NC_CACHE = build_nc()
    nc = _NC_CACHE

    in_maps = make_in_maps(x, Wv, Wo, bo)
    res = run_bass_kernel_spmd(nc, in_maps, core_ids=list(range(NCORES))).results
    return np.stack(
        [res[j]["out"].astype(np.float32) for j in range(NCORES)], axis=0
    )


# revision 21
# speedup vs baseline: 1.0503x; 1.0281x over previous
"""Trainium2 Bass kernel for nn_MultiHeadAttention_79130477461654.

The reference einsum "nhqk,nhvd->nhqd" contracts k and v independently, so
out = (sum_k softmax(energy))*(sum_s v) = broadcast(sum_s v) since softmax
rows sum to 1.  With v = split_heads(x @ Wv) and the reference's direct
(n,h,q,d)->(n,s,e) reshape, the full output reduces to

    xs[n]    = sum_s x[n,s,:]                       (1024,)
    Z[n]     = xs[n] @ Wv                           (1024,)
    WoSum    = sum_m Wo[64m+d, :]  (d=0..63)        (64, 1024)
    T[n,h,:] = Z[n][64h:64h+64] @ WoSum + bo        (16, 1024)
    out[n, 64h+r, :] = T[n,h,:]   for r in 0..63

Sharding: data parallel over batch N=8, one batch per core; weights
replicated.  All arithmetic on-device.

v13 = v2 (the empirical best at 41.0us) + one fix: v2's K=1 bias matmul
head-of-line-blocked the PE queue until ~15us waiting on the slow SWDGE
const DMA for bo.  bo is now uploaded host-pre-broadcast as a [128,1024]
tile on the SP ring (a normal 256 KB stream item), the bias matmuls are
gone, and the bias is added by DVE during the PSUM->SBUF tb8 copy.

v2 structure: all-bf16 streams; x-chunk k and Wv-chunk k adjacent on
ring k%2 so the Z accumulation chases the stream; Wo last as two
column-halves (4 staggered 512 KB sub-tiles) with the WoSum row-fold
fused into the T matmuls; Z->srow on ACT (bf16, single-pass rank-1
transposes); out as two column-half broadcast DMAs.
"""

import numpy as np

N, S, E, H, D = 8, 1024, 1024, 16, 64
NCORES = 8
P = 128  # partitions
NCHUNK = 8  # 1024 rows / 128


def build_nc():
    import concourse.bacc as bacc
    import concourse.mybir as mybir
    from concourse.tile import TileContext

    F32 = mybir.dt.float32
    BF16 = mybir.dt.bfloat16
    nc = bacc.Bacc("TRN2", target_bir_lowering=False, debug=False)

    xtd = nc.declare_dram_parameter("xT", [E, S], BF16, isOutput=False)
    wvd = nc.declare_dram_parameter("Wv", [E, E], BF16, isOutput=False)
    # Wo re-laid-out on host as two contiguous column halves: [2048, 512]
    wod = nc.declare_dram_parameter("WoH", [2 * E, E // 2], BF16, isOutput=False)
    bod = nc.declare_dram_parameter("bo128", [P, E], BF16, isOutput=False)
    i2d = nc.declare_dram_parameter("I2", [D, P], BF16, isOutput=False)
    outd = nc.declare_dram_parameter("out", [S, E], BF16, isOutput=True)

    # two HWDGE queues: SP (sync) and ACT (scalar)
    dmae = [nc.sync, nc.scalar]
    Copy = mybir.ActivationFunctionType.Copy

    with TileContext(nc) as tc:
        with (
            tc.tile_pool(name="xin", bufs=NCHUNK) as xp,
            tc.tile_pool(name="wv", bufs=NCHUNK) as wvp,
            tc.tile_pool(name="wo", bufs=4) as wop,
            tc.tile_pool(name="small", bufs=1) as sp,
            tc.tile_pool(name="psZ", bufs=1, space="PSUM") as psZ,
            tc.tile_pool(name="psS", bufs=1, space="PSUM") as psS,
            tc.tile_pool(name="psY", bufs=1, space="PSUM") as psY,
            tc.tile_pool(name="psT", bufs=1, space="PSUM") as psT,
        ):
            # I2 on the SWDGE queue (needed only at the dup matmul, slow
            # SWDGE small-transfer latency is fine for it)
            i2_sb = sp.tile([D, P], BF16)
            nc.gpsimd.dma_start(out=i2_sb[:], in_=i2d[:])
            ones18 = sp.tile([1, 8], BF16)
            nc.vector.memset(ones18[:], 1.0)

            # ---- input DMAs: x/Wv as 8 256KB chunk tiles each, chunk k of
            #      x and Wv adjacent on queue k%2 so Z-chunk matmuls fire
            #      throughout the stream; bo128 then Wo last (two column
            #      halves, each split into rb-groups 0-3 / 4-7).
            xr = xtd.rearrange("(k p) s -> k p s", p=P)
            wr = wvd.rearrange("(k p) e -> k p e", p=P)
            wor = wod.rearrange("(t rb p) c -> t p rb c", rb=4, p=P)
            xts = [None] * NCHUNK
            wvt = [None] * NCHUNK
            for k in range(NCHUNK):
                t = xp.tile([P, S], BF16, tag="xt")
                dmae[k % 2].dma_start(out=t[:], in_=xr[k])
                xts[k] = t
            for k in range(NCHUNK):
                t = wvp.tile([P, E], BF16, tag="wv")
                dmae[k % 2].dma_start(out=t[:], in_=wr[k])
                wvt[k] = t
            bo_sb = sp.tile([P, E], BF16)
            dmae[0].dma_start(out=bo_sb[:], in_=bod[:])
            wot = [None] * 4
            for i in range(4):
                # i = 0,1 -> column half A (rb 0-3, 4-7); i = 2,3 -> half B.
                # halves split across both queues so half A lands first.
                t = wop.tile([P, 4 * (E // 2)], BF16, tag="wo")
                dmae[i % 2].dma_start(
                    out=t[:].rearrange("p (rb c) -> p rb c", rb=4), in_=wor[i]
                )
                wot[i] = t

            # ---- DVE: per-chunk seq-sum of x straight to bf16 (fp32
            #      internal accumulation on DVE): xpb[p, k] = sum_s x[128k+p, s]
            xpb = sp.tile([P, NCHUNK], BF16)
            with nc.allow_low_precision(
                reason="reduces accumulate fp32 internally; bf16 only on write"
            ):
                for k in range(NCHUNK):
                    if k % 2 == 0:
                        nc.vector.tensor_reduce(
                            xpb[:, k : k + 1],
                            xts[k][:],
                            axis=mybir.AxisListType.X,
                            op=mybir.AluOpType.add,
                        )
                    else:
                        # GpSimd pre-folds odd chunks 2:1 so the serial DVE
                        # chain (4 full + 4 half reduces) ends ~2us sooner
                        fk = sp.tile([P, S // 2], BF16, tag="fold")
                        nc.gpsimd.tensor_add(
                            fk[:], xts[k][:, 0 : S // 2], xts[k][:, S // 2 : S]
                        )
                        nc.vector.tensor_reduce(
                            xpb[:, k : k + 1],
                            fk[:],
                            axis=mybir.AxisListType.X,
                            op=mybir.AluOpType.add,
                        )

            # ---- Z row (1, 1024) = xs @ Wv, accumulated chunk by chunk as
            #      the stream delivers (x_k, Wv_k); bf16 single-pass.
            ps_z = psZ.tile([1, E], F32, tag="psz")
            for k in range(NCHUNK):
                for half in range(2):
                    sl = slice(half * 512, half * 512 + 512)
                    nc.tensor.matmul(
                        ps_z[0:1, sl],
                        xpb[:, k : k + 1],
                        wvt[k][:, sl],
                        start=(k == 0),
                        stop=(k == NCHUNK - 1),
                        skip_group_check=True,
                    )

            # ---- Z -> srow (bf16, on ACT so DVE stays free and the rank-1
            #      transposes below run single-pass bf16)
            srow = sp.tile([1, E], BF16)
            for half in range(2):
                sl = slice(half * 512, half * 512 + 512)
                nc.scalar.activation(
                    srow[0:1, sl], ps_z[0:1, sl], func=Copy,
                )

            # ---- transpose dance: ps_sft[d, 8h+rr] = Z[64h+d] via 16
            #      rank-1 matmuls (rhs = ones[1,8] replicates over rr)
            ps_sft = psS.tile([D, P], F32, tag="pss")
            for h in range(H):
                nc.tensor.matmul(
                    ps_sft[:, 8 * h : 8 * h + 8],
                    srow[0:1, h * D : (h + 1) * D],
                    ones18[0:1, :],
                    start=True,
                    stop=True,
                    skip_group_check=True,
                )
            sft8 = sp.tile([D, P], BF16)
            nc.vector.tensor_copy(sft8[:], ps_sft[:])
            # dup matmul: ytx8[p, m] = sft8[p%64, m]  (I2[d,p]=1 iff d==p%64)
            ps_ytx = psY.tile([P, P], F32, tag="psy")
            nc.tensor.matmul(
                ps_ytx[:], i2_sb[:], sft8[:], start=True, stop=True,
                skip_group_check=True,
            )
            ytx8 = sp.tile([P, P], BF16)
            nc.vector.tensor_copy(ytx8[:], ps_ytx[:])

            # ---- T accumulation fused with the Wo row-fold: for column
            #      half, psT[:, half] = sum_rb ytx8 @ Wo[128rb+p, half].
            #      Chases the Wo stream tile by tile; then DVE bias-add
            #      (bf16 out) and the broadcast store
            #      out[8m + r8, half] = tb8[m, half].
            ps_t = psT.tile([P, E], F32, tag="pst")
            tb8 = sp.tile([P, E], BF16)
            outr = outd.rearrange("(m r8) e -> m r8 e", r8=8)
            for half in range(2):
                sl = slice(half * 512, half * 512 + 512)
                for i in (0, 1):
                    wt = wot[2 * half + i]
                    for rb in range(4):
                        nc.tensor.matmul(
                            ps_t[:, sl],
                            ytx8[:],
                            wt[:, rb * 512 : rb * 512 + 512],
                            start=(i == 0 and rb == 0),
                            stop=(i == 1 and rb == 3),
                            skip_group_check=True,
                        )
                nc.vector.tensor_add(tb8[:, sl], ps_t[:, sl], bo_sb[:, sl])
                dmae[half].dma_start(
                    out=outr[:, :, sl],
                    in_=tb8[:, None, sl].to_broadcast((P, 8, 512)),
                )

    nc.compile()
    return nc


_NC_CACHE = None


def make_in_maps(x, Wv, Wo, bo):
    import ml_dtypes

    BF = ml_dtypes.bfloat16
    x = np.asarray(x, dtype=np.float32)
    Wv = np.ascontiguousarray(np.asarray(Wv, dtype=np.float32).astype(BF))
    Wo = np.asarray(Wo, dtype=np.float32).astype(BF)
    WoH = np.ascontiguousarray(np.concatenate([Wo[:, :512], Wo[:, 512:]], axis=0))
    bo128 = np.ascontiguousarray(
        np.broadcast_to(np.asarray(bo, dtype=np.float32).astype(BF), (P, E))
    )
    I2 = np.zeros((D, P), dtype=BF)
    I2[np.arange(P) % D, np.arange(P)] = 1.0
    return [
        {
            "xT": np.ascontiguousarray(x[j].T.astype(BF)),
            "Wv": Wv,
            "WoH": WoH,
            "bo128": bo128,
            "I2": I2,
        }
        for j in range(NCORES)
    ]


def kernel(x, Wq=None, Wk=None, Wv=None, Wo=None, bo=None, **_unused):
    from concourse.bass_utils import run_bass_kernel_spmd

    global _NC_CACHE
    if _NC_CACHE is None:
        _NC_CACHE = build_nc()
    nc = _NC_CACHE

    in_maps = make_in_maps(x, Wv, Wo, bo)
    res = run_bass_kernel_spmd(nc, in_maps, core_ids=list(range(NCORES))).results
    return np.stack(
        [res[j]["out"].astype(np.float32) for j in range(NCORES)], axis=0
    )


# revision 22
# speedup vs baseline: 1.1165x; 1.0630x over previous
"""Trainium2 Bass kernel for nn_MultiHeadAttention_79130477461654.

The reference einsum "nhqk,nhvd->nhqd" contracts k and v independently, so
out = (sum_k softmax(energy))*(sum_s v) = broadcast(sum_s v) since softmax
rows sum to 1.  With v = split_heads(x @ Wv) and the reference's direct
(n,h,q,d)->(n,s,e) reshape, the full output reduces to

    xs[n]    = sum_s x[n,s,:]                       (1024,)
    Z[n]     = xs[n] @ Wv                           (1024,)
    WoSum    = sum_m Wo[64m+d, :]  (d=0..63)        (64, 1024)
    T[n,h,:] = Z[n][64h:64h+64] @ WoSum + bo        (16, 1024)
    out[n, 64h+r, :] = T[n,h,:]   for r in 0..63

Sharding: data parallel over batch N=8, one batch per core; weights
replicated.  All arithmetic on-device.

v13 = v2 (the empirical best at 41.0us) + one fix: v2's K=1 bias matmul
head-of-line-blocked the PE queue until ~15us waiting on the slow SWDGE
const DMA for bo.  bo is now uploaded host-pre-broadcast as a [128,1024]
tile on the SP ring (a normal 256 KB stream item), the bias matmuls are
gone, and the bias is added by DVE during the PSUM->SBUF tb8 copy.

v2 structure: all-bf16 streams; x-chunk k and Wv-chunk k adjacent on
ring k%2 so the Z accumulation chases the stream; Wo last as two
column-halves (4 staggered 512 KB sub-tiles) with the WoSum row-fold
fused into the T matmuls; Z->srow on ACT (bf16, single-pass rank-1
transposes); out as two column-half broadcast DMAs.
"""

import numpy as np

N, S, E, H, D = 8, 1024, 1024, 16, 64
NCORES = 8
P = 128  # partitions
NCHUNK = 8  # 1024 rows / 128


def build_nc():
    import concourse.bacc as bacc
    import concourse.mybir as mybir
    from concourse.tile import TileContext

    F32 = mybir.dt.float32
    BF16 = mybir.dt.bfloat16
    nc = bacc.Bacc("TRN2", target_bir_lowering=False, debug=False)

    xtd = nc.declare_dram_parameter("xT", [E, S], BF16, isOutput=False)
    wvd = nc.declare_dram_parameter("Wv", [E, E], BF16, isOutput=False)
    # Wo re-laid-out on host as two contiguous column halves: [2048, 512]
    wod = nc.declare_dram_parameter("WoH", [2 * E, E // 2], BF16, isOutput=False)
    bod = nc.declare_dram_parameter("bo128", [P, E], BF16, isOutput=False)
    i2d = nc.declare_dram_parameter("I2", [D, P], BF16, isOutput=False)
    outd = nc.declare_dram_parameter("out", [S, E], BF16, isOutput=True)

    # two HWDGE queues: SP (sync) and ACT (scalar)
    dmae = [nc.sync, nc.scalar]
    Copy = mybir.ActivationFunctionType.Copy

    with TileContext(nc) as tc:
        with (
            tc.tile_pool(name="xin", bufs=NCHUNK) as xp,
            tc.tile_pool(name="wv", bufs=NCHUNK) as wvp,
            tc.tile_pool(name="wo", bufs=4) as wop,
            tc.tile_pool(name="small", bufs=1) as sp,
            tc.tile_pool(name="psZ", bufs=1, space="PSUM") as psZ,
            tc.tile_pool(name="psS", bufs=1, space="PSUM") as psS,
            tc.tile_pool(name="psY", bufs=1, space="PSUM") as psY,
            tc.tile_pool(name="psT", bufs=1, space="PSUM") as psT,
        ):
            # I2 on the SWDGE queue (needed only at the dup matmul, slow
            # SWDGE small-transfer latency is fine for it)
            i2_sb = sp.tile([D, P], BF16)
            nc.gpsimd.dma_start(out=i2_sb[:], in_=i2d[:])
            ones18 = sp.tile([1, 8], BF16)
            nc.vector.memset(ones18[:], 1.0)

            # ---- input DMAs: x/Wv as 8 256KB chunk tiles each, chunk k of
            #      x and Wv adjacent on queue k%2 so Z-chunk matmuls fire
            #      throughout the stream; bo128 then Wo last (two column
            #      halves, each split into rb-groups 0-3 / 4-7).
            xr = xtd.rearrange("(k p) s -> k p s", p=P)
            wr = wvd.rearrange("(k p) e -> k p e", p=P)
            wor = wod.rearrange("(t rb p) c -> t p rb c", rb=4, p=P)
            xts = [None] * NCHUNK
            wvt = [None] * NCHUNK
            for k in range(NCHUNK):
                t = xp.tile([P, S], BF16, tag="xt")
                dmae[k % 2].dma_start(out=t[:], in_=xr[k])
                xts[k] = t
            for k in range(NCHUNK):
                t = wvp.tile([P, E], BF16, tag="wv")
                dmae[k % 2].dma_start(out=t[:], in_=wr[k])
                wvt[k] = t
            wot = [None] * 4
            bo_sb = sp.tile([P, E], BF16)
            for i in range(4):
                # i = 0,1 -> column half A (rb 0-3, 4-7); i = 2,3 -> half B.
                # halves split across both queues so half A lands first;
                # bo128 rides ring0 between the halves (needed only at tb8)
                # so woA is not delayed behind it.
                t = wop.tile([P, 4 * (E // 2)], BF16, tag="wo")
                dmae[i % 2].dma_start(
                    out=t[:].rearrange("p (rb c) -> p rb c", rb=4), in_=wor[i]
                )
                wot[i] = t
                if i == 1:
                    dmae[0].dma_start(out=bo_sb[:], in_=bod[:])

            # ---- DVE: per-chunk seq-sum of x straight to bf16 (fp32
            #      internal accumulation on DVE): xpb[p, k] = sum_s x[128k+p, s]
            xpb = sp.tile([P, NCHUNK], BF16)
            with nc.allow_low_precision(
                reason="reduces accumulate fp32 internally; bf16 only on write"
            ):
                for k in range(NCHUNK):
                    if k % 2 == 0:
                        nc.vector.tensor_reduce(
                            xpb[:, k : k + 1],
                            xts[k][:],
                            axis=mybir.AxisListType.X,
                            op=mybir.AluOpType.add,
                        )
                    else:
                        # GpSimd pre-folds odd chunks 2:1 so the serial DVE
                        # chain (4 full + 4 half reduces) ends ~2us sooner
                        fk = sp.tile([P, S // 2], BF16, tag="fold")
                        nc.gpsimd.tensor_add(
                            fk[:], xts[k][:, 0 : S // 2], xts[k][:, S // 2 : S]
                        )
                        nc.vector.tensor_reduce(
                            xpb[:, k : k + 1],
                            fk[:],
                            axis=mybir.AxisListType.X,
                            op=mybir.AluOpType.add,
                        )

            # ---- Z row (1, 1024) = xs @ Wv, accumulated chunk by chunk as
            #      the stream delivers (x_k, Wv_k); bf16 single-pass.
            ps_z = psZ.tile([1, E], F32, tag="psz")
            for k in range(NCHUNK):
                for half in range(2):
                    sl = slice(half * 512, half * 512 + 512)
                    nc.tensor.matmul(
                        ps_z[0:1, sl],
                        xpb[:, k : k + 1],
                        wvt[k][:, sl],
                        start=(k == 0),
                        stop=(k == NCHUNK - 1),
                        skip_group_check=True,
                    )

            # ---- Z -> srow (bf16, on ACT so DVE stays free and the rank-1
            #      transposes below run single-pass bf16)
            srow = sp.tile([1, E], BF16)
            for half in range(2):
                sl = slice(half * 512, half * 512 + 512)
                nc.scalar.activation(
                    srow[0:1, sl], ps_z[0:1, sl], func=Copy,
                )

            # ---- transpose dance: ps_sft[d, 8h+rr] = Z[64h+d] via 16
            #      rank-1 matmuls (rhs = ones[1,8] replicates over rr)
            ps_sft = psS.tile([D, P], F32, tag="pss")
            for h in range(H):
                nc.tensor.matmul(
                    ps_sft[:, 8 * h : 8 * h + 8],
                    srow[0:1, h * D : (h + 1) * D],
                    ones18[0:1, :],
                    start=True,
                    stop=True,
                    skip_group_check=True,
                )
            sft8 = sp.tile([D, P], BF16)
            nc.vector.tensor_copy(sft8[:], ps_sft[:])
            # dup matmul: ytx8[p, m] = sft8[p%64, m]  (I2[d,p]=1 iff d==p%64)
            ps_ytx = psY.tile([P, P], F32, tag="psy")
            nc.tensor.matmul(
                ps_ytx[:], i2_sb[:], sft8[:], start=True, stop=True,
                skip_group_check=True,
            )
            ytx8 = sp.tile([P, P], BF16)
            nc.vector.tensor_copy(ytx8[:], ps_ytx[:])

            # ---- T accumulation fused with the Wo row-fold: for column
            #      half, psT[:, half] = sum_rb ytx8 @ Wo[128rb+p, half].
            #      Chases the Wo stream tile by tile; then DVE bias-add
            #      (bf16 out) and the broadcast store
            #      out[8m + r8, half] = tb8[m, half].
            ps_t = psT.tile([P, E], F32, tag="pst")
            tb8 = sp.tile([P, E], BF16)
            outr = outd.rearrange("(m r8) e -> m r8 e", r8=8)
            for half in range(2):
                sl = slice(half * 512, half * 512 + 512)
                for i in (0, 1):
                    wt = wot[2 * half + i]
                    for rb in range(4):
                        nc.tensor.matmul(
                            ps_t[:, sl],
                            ytx8[:],
                            wt[:, rb * 512 : rb * 512 + 512],
                            start=(i == 0 and rb == 0),
                            stop=(i == 1 and rb == 3),
                            skip_group_check=True,
                        )
                nc.vector.tensor_add(tb8[:, sl], ps_t[:, sl], bo_sb[:, sl])
                dmae[half].dma_start(
                    out=outr[:, :, sl],
                    in_=tb8[:, None, sl].to_broadcast((P, 8, 512)),
                )

    nc.compile()
    return nc


_NC_CACHE = None


def make_in_maps(x, Wv, Wo, bo):
    import ml_dtypes

    BF = ml_dtypes.bfloat16
    x = np.asarray(x, dtype=np.float32)
    Wv = np.ascontiguousarray(np.asarray(Wv, dtype=np.float32).astype(BF))
    Wo = np.asarray(Wo, dtype=np.float32).astype(BF)
    WoH = np.ascontiguousarray(np.concatenate([Wo[:, :512], Wo[:, 512:]], axis=0))
    bo128 = np.ascontiguousarray(
        np.broadcast_to(np.asarray(bo, dtype=np.float32).astype(BF), (P, E))
    )
    I2 = np.zeros((D, P), dtype=BF)
    I2[np.arange(P) % D, np.arange(P)] = 1.0
    return [
        {
            "xT": np.ascontiguousarray(x[j].T.astype(BF)),
            "Wv": Wv,
            "WoH": WoH,
            "bo128": bo128,
            "I2": I2,
        }
        for j in range(NCORES)
    ]


def kernel(x, Wq=None, Wk=None, Wv=None, Wo=None, bo=None, **_unused):
    from concourse.bass_utils import run_bass_kernel_spmd

    global _NC_CACHE
    if _NC_CACHE is None:
        _NC_CACHE = build_nc()
    nc = _NC_CACHE

    in_maps = make_in_maps(x, Wv, Wo, bo)
    res = run_bass_kernel_spmd(nc, in_maps, core_ids=list(range(NCORES))).results
    return np.stack(
        [res[j]["out"].astype(np.float32) for j in range(NCORES)], axis=0
    )
